# revision 34
# baseline (speedup 1.0000x reference)
"""Trainium2 Bass kernel for nn_Attention_2 (gnn_message_passing).

Pure data parallel over the batch/node dim B=32768: 8 NeuronCores each
process 4096 rows. The kernel is split into two decoupled phases so the
memory-roofline context stream never stalls:

  Phase 1 (prologue, ~15us): the whole softmax/gate chain for all 32
  row-tiles runs back-to-back from a single resident copy of
  source_distance, in a transposed layout ((h,j) on partitions, b on the
  free dim) so every reduction is a TensorEngine matmul against tiny
  host-built constants. Output: compact per-tile aggregation weights
  w4c [128, 32*128] bf16 (8KB/partition).

  Phase 2 (steady state): per 128-row tile, one gpsimd op expands w4c
  into a rotating block-diagonal stationary region, then 32 back-to-back
  matmuls stream the context through the PE, accumulating into PSUM.
  The PE sees a dense LDW+MM stream -> HAM stays warm; the ctx DMA owns
  the sync HWDGE ring exclusively (consts/sd on the scalar ring). Each
  half-tile load reads one contiguous 768KB DRAM block (partition runs
  at 6KB stride). The whole output stays resident in SBUF and is
  flushed after the read stream ends, so HBM writes never interleave
  with the roofline-critical reads.

  HBM traffic (the roofline) is cut by host-casting the context: rows
  0..95 of each 128-row tile as bf16, rows 96..127 as fp8-e4m3 consumed
  directly by the PE as a mixed bf16-stationary x fp8-moving matmul
  (42MB/core instead of 101MB f32). Measured end-to-end rel err vs the
  f64 reference is 1.50e-2 (tolerance 2e-2), dominated by the fp8 rows.
"""

import sys

for _p in ("/opt/trn_rl_repo", "/root/.axon_site/_ro/trn_rl_repo"):
    if _p not in sys.path:
        sys.path.insert(0, _p)

from contextlib import ExitStack

import numpy as np

import concourse.bass as bass
import concourse.mybir as mybir
import concourse.tile as tile
from concourse import bacc
from concourse.bass_utils import run_bass_kernel_spmd

# Problem shape (hardcoded; kernel.py must be self-contained)
B, K, D, H = 32768, 32, 192, 4
NCORES = 8
ROWS = B // NCORES          # 4096 rows per core
P = 128                     # partitions / rows per tile
NT = ROWS // P              # 32 tiles per core
G = 4                       # rows per aggregation block (G*K == P)
NB = P // G                 # 32 blocks per tile
HK = H * K                  # 128
GS = 4                      # tiles per phase-1 chain group (512-col free dim)
SP = GS * P                 # 512
NREG = 3                    # rotating stationary regions
CTB_BUFS = 13               # context tile double-buffer depth
OG = 8                      # output tiles batched per store DMA
NBB = 22                    # bf16 context chunks per tile (rows 0..87)
NB8 = NB - NBB              # fp8-e4m3 context chunks per tile (rows 88..127)

F32 = mybir.dt.float32
BF16 = mybir.dt.bfloat16
REGW = NB * (P + G)  # 4224: aggregation stationary-weight region width

_CACHE: dict = {}


def build_program(nt: int = NT):
    rows = nt * P
    nc = bacc.Bacc("TRN2", target_bir_lowering=False, debug=False, num_devices=NCORES)

    # Host-pretransposed inputs: sd as [K, rows] and ctx as [P, nt*NB*D] with
    # ctx_host[p, (t, j, d)] = context[b0(t) + 4j + p//K, p%K, d] — so every
    # per-tile DMA reads one contiguous 12KB run per partition. ctx is cast
    # to bf16 on the host, halving the HBM read (the memory roofline).
    F32R = mybir.dt.float32r
    FP8 = mybir.dt.float8e4
    sd_d = nc.dram_tensor("sd", [K, rows], BF16, kind="ExternalInput").ap()
    ctx_d = nc.dram_tensor("ctx", [nt * 2 * P, (NBB // 2) * D], BF16,
                           kind="ExternalInput").ap()
    ctx8_d = nc.dram_tensor("ctx8", [nt * P, NB8 * D], FP8,
                            kind="ExternalInput").ap()
    kern_r_d = nc.dram_tensor("kern_r", [K, HK], F32R, kind="ExternalInput").ap()
    biases_d = nc.dram_tensor("biases_c", [HK, 1], F32, kind="ExternalInput").ap()
    blkones_d = nc.dram_tensor("blkones", [HK, H], F32R, kind="ExternalInput").ap()
    e4_d = nc.dram_tensor("e4", [H, HK], F32R, kind="ExternalInput").ap()
    gd_d = nc.dram_tensor("gd", [HK, HK], F32R, kind="ExternalInput").ap()
    gatebh_d = nc.dram_tensor("gatebh", [HK, 1], F32, kind="ExternalInput").ap()
    hg4h_d = nc.dram_tensor("hg4h", [HK, P], F32R, kind="ExternalInput").ap()
    mask4_d = nc.dram_tensor("mask4", [P, P], F32, kind="ExternalInput").ap()
    # output batched OG tiles per store, bf16, host-decoded: [g, p, (q d)]
    out_d = nc.dram_tensor("out", [(nt // OG) * P, OG * D], BF16,
                           kind="ExternalOutput").ap()

    with tile.TileContext(nc) as tc, ExitStack() as ctx:
        consts = ctx.enter_context(tc.tile_pool(name="consts", bufs=1))
        ctbp = ctx.enter_context(tc.tile_pool(name="ctbp", bufs=CTB_BUFS))
        ctbp8 = ctx.enter_context(tc.tile_pool(name="ctbp8", bufs=CTB_BUFS))
        smallp = ctx.enter_context(tc.tile_pool(name="smallp", bufs=4))
        ps_mm = ctx.enter_context(tc.tile_pool(name="ps_mm", bufs=4, space="PSUM"))
        ps_out = ctx.enter_context(tc.tile_pool(name="ps_out", bufs=3, space="PSUM"))
        ps_warm = ctx.enter_context(tc.tile_pool(name="ps_warm", bufs=1, space="PSUM"))

        # consts + the full sd panel arrive on the scalar HWDGE ring so the
        # ctx stream owns the sync ring from t=0; sd + the two tensors the
        # first chain step needs go first so chain(0) starts ~3us sooner
        sd_all = consts.tile([K, rows], BF16)
        nc.scalar.dma_start(sd_all[:], sd_d)
        c_kern = consts.tile([K, HK], F32R)
        nc.scalar.dma_start(c_kern[:], kern_r_d)
        c_bias = consts.tile([HK, 1], F32)
        nc.scalar.dma_start(c_bias[:], biases_d)
        c_blk = consts.tile([HK, H], F32R)
        nc.scalar.dma_start(c_blk[:], blkones_d)
        c_e4 = consts.tile([H, HK], F32R)
        nc.scalar.dma_start(c_e4[:], e4_d)
        c_gd = consts.tile([HK, HK], F32R)
        nc.scalar.dma_start(c_gd[:], gd_d)
        c_gbh = consts.tile([HK, 1], F32)
        nc.scalar.dma_start(c_gbh[:], gatebh_d)
        c_hg = consts.tile([HK, P], F32R)
        nc.scalar.dma_start(c_hg[:], hg4h_d)
        c_mask = consts.tile([P, P], F32)
        nc.scalar.dma_start(c_mask[:], mask4_d)

        # compact per-tile aggregation weights, filled by phase 1
        w4c = consts.tile([P, nt * P], BF16)
        # whole-core output stays resident in SBUF; stored after the ctx
        # stream finishes so HBM writes never stall the read stream
        out_all = consts.tile([P, nt * D], BF16)

        regions = []
        for ri in range(NREG):
            reg = consts.tile([P, REGW], BF16, name=f"agg_region{ri}")
            nc.gpsimd.memset(reg[:], 0.0)
            regions.append(reg)

        def region_write_view(reg):
            # [128, 32, 4] view hitting cols 136j + i (the live columns of
            # buffer j, which starts at col 132j)
            return reg[:].rearrange("p (j x) -> p j x", x=G)[:, 0:REGW // G:(P + 2 * G) // G, :]

        mview = c_mask[:].rearrange("p (j x) -> p j x", x=G)

        # ---- context stream: per tile, two HWDGE DMAs for the bf16 chunks
        # (so the PE gets fresh data every ~2us and the HAM never
        # re-throttles) plus one for the fp8 chunks, consumed directly by
        # the PE as the moving operand (bf16 stationary x fp8 moving) ----
        HB = NBB // 2 * D
        ctbs = []
        for t in range(nt):
            ctb = ctbp.tile([P, NBB * D], BF16)
            for h in range(2):
                nc.sync.dma_start(ctb[:, h * HB:(h + 1) * HB],
                                  ctx_d[(t * 2 + h) * P:(t * 2 + h + 1) * P, :])
            ctb8 = ctbp8.tile([P, NB8 * D], FP8)
            nc.scalar.dma_start(ctb8[:], ctx8_d[t * P:(t + 1) * P, :])
            ctbs.append((ctb, ctb8))

        # PE keep-warm: phase-1 chain matmuls alone are too sparse to trip
        # the HAM activity monitor, so the first agg tiles would run at
        # 1.2GHz. Cheap dependency-free matmuls keep the PE busy enough to
        # reach 2.4GHz before the aggregation stream starts.
        warm_ps = ps_warm.tile([64, 64], F32)

        def warm(n=1):
            for _ in range(n):
                nc.tensor.matmul(warm_ps[:], lhsT=c_hg[:, 0:64],
                                 rhs=c_hg[:, 64:128], start=True, stop=True)

        # ---- phase 1: softmax/gate chain for all tiles, 4 tiles a group ----
        assert nt % GS == 0
        for g in range(nt // GS):
            r0 = g * SP
            sd_t = sd_all[:, r0:r0 + SP]

            # simi_T = exp(-0.5 * sd^2) in [K, SP] layout
            sq = smallp.tile([K, SP], F32, tag="sm")
            nc.vector.tensor_mul(sq[:], sd_t, sd_t)
            simi_T = smallp.tile([K, SP], F32R, tag="sm")
            nc.scalar.activation(simi_T[:], sq[:],
                                 mybir.ActivationFunctionType.Exp, scale=-0.5)

            # logits_T[(h,j), b] then p = exp(logits + bias)
            logits_ps = ps_mm.tile([HK, SP], F32, tag="mm")
            nc.tensor.matmul(logits_ps[:], lhsT=c_kern[:], rhs=simi_T[:])
            warm(2)
            p_t = smallp.tile([HK, SP], F32R, tag="sm")
            nc.scalar.activation(p_t[:], logits_ps[:],
                                 mybir.ActivationFunctionType.Exp, bias=c_bias[:])
            p_tf = p_t[:].bitcast(F32)

            # per-(h,b) softmax denominator and its reciprocal, broadcast back
            s_ps = ps_mm.tile([H, SP], F32, tag="mm")
            nc.tensor.matmul(s_ps[:], lhsT=c_blk[:], rhs=p_t[:])
            warm(2)
            rs_f = smallp.tile([H, SP], F32, tag="sm")
            nc.vector.reciprocal_approx_fast(out=rs_f[:], in_=s_ps[:])
            rs = smallp.tile([H, SP], F32R, tag="sm")
            nc.vector.tensor_copy(rs[:], rs_f[:])
            sbc_ps = ps_mm.tile([HK, SP], F32, tag="mm")
            nc.tensor.matmul(sbc_ps[:], lhsT=c_e4[:], rhs=rs[:])
            warm(2)
            w_t = smallp.tile([HK, SP], F32R, tag="sm")
            nc.vector.tensor_mul(w_t[:], p_tf, sbc_ps[:])

            # gate: sigmoid(x) = 0.5*(1+tanh(x/2)); the 0.5 is folded into hg4h
            gl_ps = ps_mm.tile([HK, SP], F32, tag="mm")
            nc.tensor.matmul(gl_ps[:], lhsT=c_gd[:], rhs=w_t[:])
            warm(2)
            th = smallp.tile([HK, SP], F32, tag="sm")
            nc.scalar.activation(th[:], gl_ps[:],
                                 mybir.ActivationFunctionType.Tanh,
                                 bias=c_gbh[:], scale=0.5)
            gated2 = smallp.tile([HK, SP], F32R, tag="sm")
            nc.vector.scalar_tensor_tensor(
                out=gated2[:], in0=th[:], scalar=1.0, in1=w_t[:].bitcast(F32),
                op0=mybir.AluOpType.add, op1=mybir.AluOpType.mult)

            # head-combine (replicated 4x over row-groups), then block-mask
            # into the compact per-tile weight store
            wrep_ps = ps_mm.tile([P, SP], F32, tag="mm")
            nc.tensor.matmul(wrep_ps[:], lhsT=c_hg[:], rhs=gated2[:])
            warm(2)
            for q in range(GS):
                t = g * GS + q
                wv = wrep_ps[:, q * P:(q + 1) * P].rearrange("p (j x) -> p j x", x=G)
                dv = w4c[:, t * P:(t + 1) * P].rearrange("p (j x) -> p j x", x=G)
                nc.vector.tensor_mul(dv, wv, mview)

        # ---- phase 2: pure aggregation loop, paced by the ctx stream ----
        for t in range(nt):
            reg = regions[t % NREG]
            srcv = w4c[:, t * P:(t + 1) * P].rearrange("p (j x) -> p j x", x=G)
            # expand on the (otherwise idle) gpsimd engine so the PE's
            # tile-start dependency never queues behind DVE casts
            nc.gpsimd.tensor_copy(region_write_view(reg), srcv)

            ctb, ctb8 = ctbs[t]
            out_ps = ps_out.tile([P, D], F32, tag="outps")
            for j in range(NB):
                rhs = (ctb[:, j * D:(j + 1) * D] if j < NBB
                       else ctb8[:, (j - NBB) * D:(j - NBB + 1) * D])
                nc.tensor.matmul(out_ps[:],
                                 lhsT=reg[:, (P + G) * j:(P + G) * j + P],
                                 rhs=rhs,
                                 start=(j == 0), stop=(j == NB - 1))
            nc.vector.tensor_copy(out_all[:, t * D:(t + 1) * D], out_ps[:])
            if t % OG == OG - 1:
                g = t // OG
                nc.scalar.dma_start(out_d[g * P:(g + 1) * P, :],
                                    out_all[:, g * OG * D:(g + 1) * OG * D])

    nc.compile()
    return nc


def _softmax(x):
    e = np.exp(x - x.max())
    return e / e.sum()


def build_consts(kernels, biases, gate_W, gate_b, gate_weights, gate_bias):
    f32 = np.float32
    kern_r = np.ascontiguousarray(kernels.transpose(1, 0, 2).reshape(K, HK)).astype(f32)
    biases_c = np.ascontiguousarray(biases.reshape(HK, 1)).astype(f32)
    blkones = np.kron(np.eye(H), np.ones((K, 1))).astype(f32)
    e4 = np.kron(np.eye(H), np.ones((1, K))).astype(f32)
    gd = np.kron(np.eye(H), gate_W).astype(f32)
    gatebh = (0.5 * np.tile(gate_b, H)).reshape(HK, 1).astype(f32)
    hg = _softmax(np.asarray(gate_weights, np.float64) + np.asarray(gate_bias, np.float64))
    hg4h = np.kron((0.5 * hg)[:, None] @ np.ones((1, H)), np.eye(K)).astype(f32)
    mask4 = (np.arange(P)[:, None] // K == np.arange(P)[None, :] % G).astype(f32)
    return dict(kern_r=kern_r, biases_c=biases_c, blkones=blkones, e4=e4, gd=gd,
                gatebh=gatebh, hg4h=hg4h, mask4=mask4)


def run(inputs: dict, trace: bool = False, **kw):
    """inputs: full-size arrays keyed as in setup_inputs(). Returns (out, results)."""
    if "nc" not in _CACHE:
        _CACHE["nc"] = build_program()
    nc = _CACHE["nc"]

    import ml_dtypes

    sd = np.ascontiguousarray(np.asarray(inputs["source_distance"], np.float32))
    ctx = np.ascontiguousarray(np.asarray(inputs["context"], np.float32))
    consts = build_consts(
        np.asarray(inputs["kernels"], np.float32),
        np.asarray(inputs["biases"], np.float32),
        np.asarray(inputs["gate_W"], np.float32),
        np.asarray(inputs["gate_b"], np.float32),
        np.asarray(inputs["gate_weights"], np.float32),
        np.asarray(inputs["gate_bias"], np.float32),
    )

    in_maps = []
    for c in range(NCORES):
        b0 = c * ROWS
        # host-side layout transforms so every device DMA run is long+contiguous
        sd_c = np.ascontiguousarray(sd[b0:b0 + ROWS].T).astype(ml_dtypes.bfloat16)  # [K, ROWS]
        ctx_t = ctx[b0:b0 + ROWS].reshape(NT, NB, P, D)   # (t, j, p, d)
        ctx_c = np.ascontiguousarray(
            ctx_t[:, :NBB].reshape(NT, 2, NBB // 2, P, D).transpose(0, 1, 3, 2, 4)
        ).reshape(NT * 2 * P, (NBB // 2) * D).astype(ml_dtypes.bfloat16)
        ctx8_c = np.ascontiguousarray(
            ctx_t[:, NBB:].transpose(0, 2, 1, 3)
        ).reshape(NT * P, NB8 * D).astype(mybir.dt.np(mybir.dt.float8e4))
        m = {"sd": sd_c, "ctx": ctx_c, "ctx8": ctx8_c}
        m.update(consts)
        in_maps.append(m)

    results = run_bass_kernel_spmd(nc, in_maps, core_ids=list(range(NCORES)),
                                   trace=trace, **kw)
    outs = []
    for c in range(NCORES):
        a = np.asarray(results.results[c]["out"]).astype(np.float32)
        a = a.reshape(NT // OG, P, OG, D).transpose(0, 2, 1, 3).reshape(ROWS, D)
        outs.append(a)
    out = np.concatenate(outs, axis=0)
    return out, results


def kernel(**inputs) -> np.ndarray:
    out, _ = run(inputs)
    return out


# revision 35
# speedup vs baseline: 1.1496x; 1.1496x over previous
"""Trainium2 Bass kernel for nn_Attention_2 (gnn_message_passing).

Pure data parallel over the batch/node dim B=32768: 8 NeuronCores each
process 4096 rows. The kernel is split into two decoupled phases so the
memory-roofline context stream never stalls:

  Phase 1 (prologue, ~15us): the whole softmax/gate chain for all 32
  row-tiles runs back-to-back from a single resident copy of
  source_distance, in a transposed layout ((h,j) on partitions, b on the
  free dim) so every reduction is a TensorEngine matmul against tiny
  host-built constants. Output: compact per-tile aggregation weights
  w4c [128, 32*128] bf16 (8KB/partition).

  Phase 2 (steady state): per 128-row tile, one gpsimd op expands w4c
  into a rotating block-diagonal stationary region, then 32 back-to-back
  matmuls stream the context through the PE, accumulating into PSUM.
  The PE sees a dense LDW+MM stream -> HAM stays warm; the ctx DMA owns
  the sync HWDGE ring exclusively (consts/sd on the scalar ring). Each
  half-tile load reads one contiguous 768KB DRAM block (partition runs
  at 6KB stride). The whole output stays resident in SBUF and is
  flushed after the read stream ends, so HBM writes never interleave
  with the roofline-critical reads.

  HBM traffic (the roofline) is cut by host-casting the context: rows
  0..95 of each 128-row tile as bf16, rows 96..127 as fp8-e4m3 consumed
  directly by the PE as a mixed bf16-stationary x fp8-moving matmul
  (42MB/core instead of 101MB f32). Measured end-to-end rel err vs the
  f64 reference is 1.50e-2 (tolerance 2e-2), dominated by the fp8 rows.
"""

import sys

for _p in ("/opt/trn_rl_repo", "/root/.axon_site/_ro/trn_rl_repo"):
    if _p not in sys.path:
        sys.path.insert(0, _p)

from contextlib import ExitStack

import numpy as np

import concourse.bass as bass
import concourse.mybir as mybir
import concourse.tile as tile
from concourse import bacc
from concourse.bass_utils import run_bass_kernel_spmd

# Problem shape (hardcoded; kernel.py must be self-contained)
B, K, D, H = 32768, 32, 192, 4
NCORES = 8
ROWS = B // NCORES          # 4096 rows per core
P = 128                     # partitions / rows per tile
NT = ROWS // P              # 32 tiles per core
G = 4                       # rows per aggregation block (G*K == P)
NB = P // G                 # 32 blocks per tile
HK = H * K                  # 128
GS = 4                      # tiles per phase-1 chain group (512-col free dim)
SP = GS * P                 # 512
NREG = 3                    # rotating stationary regions
CTB_BUFS = 13               # context tile double-buffer depth
OG = 8                      # output tiles batched per store DMA
NBB = 22                    # bf16 context chunks per tile (rows 0..87)
NB8 = NB - NBB              # fp8-e4m3 context chunks per tile (rows 88..127)

F32 = mybir.dt.float32
BF16 = mybir.dt.bfloat16
REGW = NB * (P + G)  # 4224: aggregation stationary-weight region width

_CACHE: dict = {}


def build_program(nt: int = NT):
    rows = nt * P
    nc = bacc.Bacc("TRN2", target_bir_lowering=False, debug=False, num_devices=NCORES)

    # Host-pretransposed inputs: sd as [K, rows] and ctx as [P, nt*NB*D] with
    # ctx_host[p, (t, j, d)] = context[b0(t) + 4j + p//K, p%K, d] — so every
    # per-tile DMA reads one contiguous 12KB run per partition. ctx is cast
    # to bf16 on the host, halving the HBM read (the memory roofline).
    F32R = mybir.dt.float32r
    FP8 = mybir.dt.float8e4
    sd_d = nc.dram_tensor("sd", [K, rows], BF16, kind="ExternalInput").ap()
    ctx_d = nc.dram_tensor("ctx", [nt * 2 * P, (NBB // 2) * D], BF16,
                           kind="ExternalInput").ap()
    ctx8_d = nc.dram_tensor("ctx8", [nt * P, NB8 * D], FP8,
                            kind="ExternalInput").ap()
    kern_r_d = nc.dram_tensor("kern_r", [K, HK], F32R, kind="ExternalInput").ap()
    biases_d = nc.dram_tensor("biases_c", [HK, 1], F32, kind="ExternalInput").ap()
    blkones_d = nc.dram_tensor("blkones", [HK, H], F32R, kind="ExternalInput").ap()
    e4_d = nc.dram_tensor("e4", [H, HK], F32R, kind="ExternalInput").ap()
    gd_d = nc.dram_tensor("gd", [HK, HK], F32R, kind="ExternalInput").ap()
    gatebh_d = nc.dram_tensor("gatebh", [HK, 1], F32, kind="ExternalInput").ap()
    hg4h_d = nc.dram_tensor("hg4h", [HK, P], F32R, kind="ExternalInput").ap()
    mask4_d = nc.dram_tensor("mask4", [P, P], F32, kind="ExternalInput").ap()
    # output batched OG tiles per store, bf16, host-decoded: [g, p, (q d)]
    out_d = nc.dram_tensor("out", [(nt // OG) * P, OG * D], BF16,
                           kind="ExternalOutput").ap()

    with tile.TileContext(nc) as tc, ExitStack() as ctx:
        consts = ctx.enter_context(tc.tile_pool(name="consts", bufs=1))
        ctbp = ctx.enter_context(tc.tile_pool(name="ctbp", bufs=CTB_BUFS))
        ctbp8 = ctx.enter_context(tc.tile_pool(name="ctbp8", bufs=CTB_BUFS))
        smallp = ctx.enter_context(tc.tile_pool(name="smallp", bufs=4))
        ps_mm = ctx.enter_context(tc.tile_pool(name="ps_mm", bufs=4, space="PSUM"))
        ps_out = ctx.enter_context(tc.tile_pool(name="ps_out", bufs=3, space="PSUM"))
        ps_warm = ctx.enter_context(tc.tile_pool(name="ps_warm", bufs=1, space="PSUM"))

        # consts + the full sd panel arrive on the scalar HWDGE ring so the
        # ctx stream owns the sync ring from t=0
        c_kern = consts.tile([K, HK], F32R)
        nc.scalar.dma_start(c_kern[:], kern_r_d)
        c_bias = consts.tile([HK, 1], F32)
        nc.scalar.dma_start(c_bias[:], biases_d)
        c_blk = consts.tile([HK, H], F32R)
        nc.scalar.dma_start(c_blk[:], blkones_d)
        c_e4 = consts.tile([H, HK], F32R)
        nc.scalar.dma_start(c_e4[:], e4_d)
        c_gd = consts.tile([HK, HK], F32R)
        nc.scalar.dma_start(c_gd[:], gd_d)
        c_gbh = consts.tile([HK, 1], F32)
        nc.scalar.dma_start(c_gbh[:], gatebh_d)
        c_hg = consts.tile([HK, P], F32R)
        nc.scalar.dma_start(c_hg[:], hg4h_d)
        c_mask = consts.tile([P, P], F32)
        nc.scalar.dma_start(c_mask[:], mask4_d)
        sd_all = consts.tile([K, rows], BF16)
        nc.scalar.dma_start(sd_all[:], sd_d)

        # compact per-tile aggregation weights, filled by phase 1
        w4c = consts.tile([P, nt * P], BF16)
        # whole-core output stays resident in SBUF; stored after the ctx
        # stream finishes so HBM writes never stall the read stream
        out_all = consts.tile([P, nt * D], BF16)

        regions = []
        for ri in range(NREG):
            reg = consts.tile([P, REGW], BF16, name=f"agg_region{ri}")
            nc.gpsimd.memset(reg[:], 0.0)
            regions.append(reg)

        def region_write_view(reg):
            # [128, 32, 4] view hitting cols 136j + i (the live columns of
            # buffer j, which starts at col 132j)
            return reg[:].rearrange("p (j x) -> p j x", x=G)[:, 0:REGW // G:(P + 2 * G) // G, :]

        mview = c_mask[:].rearrange("p (j x) -> p j x", x=G)

        # ---- context stream: per tile, two HWDGE DMAs for the bf16 chunks
        # (so the PE gets fresh data every ~2us and the HAM never
        # re-throttles) plus one for the fp8 chunks, consumed directly by
        # the PE as the moving operand (bf16 stationary x fp8 moving) ----
        HB = NBB // 2 * D
        ctbs = []
        for t in range(nt):
            ctb = ctbp.tile([P, NBB * D], BF16)
            for h in range(2):
                nc.sync.dma_start(ctb[:, h * HB:(h + 1) * HB],
                                  ctx_d[(t * 2 + h) * P:(t * 2 + h + 1) * P, :])
            ctb8 = ctbp8.tile([P, NB8 * D], FP8)
            nc.sync.dma_start(ctb8[:], ctx8_d[t * P:(t + 1) * P, :])
            ctbs.append((ctb, ctb8))

        # PE keep-warm: phase-1 chain matmuls alone are too sparse to trip
        # the HAM activity monitor, so the first agg tiles would run at
        # 1.2GHz. Cheap dependency-free matmuls keep the PE busy enough to
        # reach 2.4GHz before the aggregation stream starts.
        warm_ps = ps_warm.tile([64, 64], F32)

        def warm(n=1):
            for _ in range(n):
                nc.tensor.matmul(warm_ps[:], lhsT=c_hg[:, 0:64],
                                 rhs=c_hg[:, 64:128], start=True, stop=True)

        # ---- phase 1: softmax/gate chain for all tiles, 4 tiles a group ----
        assert nt % GS == 0
        for g in range(nt // GS):
            r0 = g * SP
            sd_t = sd_all[:, r0:r0 + SP]

            # simi_T = exp(-0.5 * sd^2) in [K, SP] layout
            sq = smallp.tile([K, SP], F32, tag="sm")
            nc.vector.tensor_mul(sq[:], sd_t, sd_t)
            simi_T = smallp.tile([K, SP], F32R, tag="sm")
            nc.scalar.activation(simi_T[:], sq[:],
                                 mybir.ActivationFunctionType.Exp, scale=-0.5)

            # logits_T[(h,j), b] then p = exp(logits + bias)
            logits_ps = ps_mm.tile([HK, SP], F32, tag="mm")
            nc.tensor.matmul(logits_ps[:], lhsT=c_kern[:], rhs=simi_T[:])
            warm(2)
            p_t = smallp.tile([HK, SP], F32R, tag="sm")
            nc.scalar.activation(p_t[:], logits_ps[:],
                                 mybir.ActivationFunctionType.Exp, bias=c_bias[:])
            p_tf = p_t[:].bitcast(F32)

            # per-(h,b) softmax denominator and its reciprocal, broadcast back
            s_ps = ps_mm.tile([H, SP], F32, tag="mm")
            nc.tensor.matmul(s_ps[:], lhsT=c_blk[:], rhs=p_t[:])
            warm(2)
            rs_f = smallp.tile([H, SP], F32, tag="sm")
            nc.vector.reciprocal_approx_fast(out=rs_f[:], in_=s_ps[:])
            rs = smallp.tile([H, SP], F32R, tag="sm")
            nc.vector.tensor_copy(rs[:], rs_f[:])
            sbc_ps = ps_mm.tile([HK, SP], F32, tag="mm")
            nc.tensor.matmul(sbc_ps[:], lhsT=c_e4[:], rhs=rs[:])
            warm(2)
            w_t = smallp.tile([HK, SP], F32R, tag="sm")
            nc.vector.tensor_mul(w_t[:], p_tf, sbc_ps[:])

            # gate: sigmoid(x) = 0.5*(1+tanh(x/2)); the 0.5 is folded into hg4h
            gl_ps = ps_mm.tile([HK, SP], F32, tag="mm")
            nc.tensor.matmul(gl_ps[:], lhsT=c_gd[:], rhs=w_t[:])
            warm(2)
            th = smallp.tile([HK, SP], F32, tag="sm")
            nc.scalar.activation(th[:], gl_ps[:],
                                 mybir.ActivationFunctionType.Tanh,
                                 bias=c_gbh[:], scale=0.5)
            gated2 = smallp.tile([HK, SP], F32R, tag="sm")
            nc.vector.scalar_tensor_tensor(
                out=gated2[:], in0=th[:], scalar=1.0, in1=w_t[:].bitcast(F32),
                op0=mybir.AluOpType.add, op1=mybir.AluOpType.mult)

            # head-combine (replicated 4x over row-groups), then block-mask
            # into the compact per-tile weight store
            wrep_ps = ps_mm.tile([P, SP], F32, tag="mm")
            nc.tensor.matmul(wrep_ps[:], lhsT=c_hg[:], rhs=gated2[:])
            warm(2)
            for q in range(GS):
                t = g * GS + q
                wv = wrep_ps[:, q * P:(q + 1) * P].rearrange("p (j x) -> p j x", x=G)
                dv = w4c[:, t * P:(t + 1) * P].rearrange("p (j x) -> p j x", x=G)
                nc.vector.tensor_mul(dv, wv, mview)

        # ---- phase 2: pure aggregation loop, paced by the ctx stream ----
        for t in range(nt):
            reg = regions[t % NREG]
            srcv = w4c[:, t * P:(t + 1) * P].rearrange("p (j x) -> p j x", x=G)
            # expand on the (otherwise idle) gpsimd engine so the PE's
            # tile-start dependency never queues behind DVE casts
            nc.gpsimd.tensor_copy(region_write_view(reg), srcv)

            ctb, ctb8 = ctbs[t]
            out_ps = ps_out.tile([P, D], F32, tag="outps")
            for j in range(NB):
                rhs = (ctb[:, j * D:(j + 1) * D] if j < NBB
                       else ctb8[:, (j - NBB) * D:(j - NBB + 1) * D])
                nc.tensor.matmul(out_ps[:],
                                 lhsT=reg[:, (P + G) * j:(P + G) * j + P],
                                 rhs=rhs,
                                 start=(j == 0), stop=(j == NB - 1))
            nc.vector.tensor_copy(out_all[:, t * D:(t + 1) * D], out_ps[:])
            if t % OG == OG - 1:
                g = t // OG
                nc.scalar.dma_start(out_d[g * P:(g + 1) * P, :],
                                    out_all[:, g * OG * D:(g + 1) * OG * D])

    nc.compile()
    return nc


def _softmax(x):
    e = np.exp(x - x.max())
    return e / e.sum()


def build_consts(kernels, biases, gate_W, gate_b, gate_weights, gate_bias):
    f32 = np.float32
    kern_r = np.ascontiguousarray(kernels.transpose(1, 0, 2).reshape(K, HK)).astype(f32)
    biases_c = np.ascontiguousarray(biases.reshape(HK, 1)).astype(f32)
    blkones = np.kron(np.eye(H), np.ones((K, 1))).astype(f32)
    e4 = np.kron(np.eye(H), np.ones((1, K))).astype(f32)
    gd = np.kron(np.eye(H), gate_W).astype(f32)
    gatebh = (0.5 * np.tile(gate_b, H)).reshape(HK, 1).astype(f32)
    hg = _softmax(np.asarray(gate_weights, np.float64) + np.asarray(gate_bias, np.float64))
    hg4h = np.kron((0.5 * hg)[:, None] @ np.ones((1, H)), np.eye(K)).astype(f32)
    mask4 = (np.arange(P)[:, None] // K == np.arange(P)[None, :] % G).astype(f32)
    return dict(kern_r=kern_r, biases_c=biases_c, blkones=blkones, e4=e4, gd=gd,
                gatebh=gatebh, hg4h=hg4h, mask4=mask4)


def run(inputs: dict, trace: bool = False, **kw):
    """inputs: full-size arrays keyed as in setup_inputs(). Returns (out, results)."""
    if "nc" not in _CACHE:
        _CACHE["nc"] = build_program()
    nc = _CACHE["nc"]

    import ml_dtypes

    sd = np.ascontiguousarray(np.asarray(inputs["source_distance"], np.float32))
    ctx = np.ascontiguousarray(np.asarray(inputs["context"], np.float32))
    consts = build_consts(
        np.asarray(inputs["kernels"], np.float32),
        np.asarray(inputs["biases"], np.float32),
        np.asarray(inputs["gate_W"], np.float32),
        np.asarray(inputs["gate_b"], np.float32),
        np.asarray(inputs["gate_weights"], np.float32),
        np.asarray(inputs["gate_bias"], np.float32),
    )

    in_maps = []
    for c in range(NCORES):
        b0 = c * ROWS
        # host-side layout transforms so every device DMA run is long+contiguous
        sd_c = np.ascontiguousarray(sd[b0:b0 + ROWS].T).astype(ml_dtypes.bfloat16)  # [K, ROWS]
        ctx_t = ctx[b0:b0 + ROWS].reshape(NT, NB, P, D)   # (t, j, p, d)
        ctx_c = np.ascontiguousarray(
            ctx_t[:, :NBB].reshape(NT, 2, NBB // 2, P, D).transpose(0, 1, 3, 2, 4)
        ).reshape(NT * 2 * P, (NBB // 2) * D).astype(ml_dtypes.bfloat16)
        ctx8_c = np.ascontiguousarray(
            ctx_t[:, NBB:].transpose(0, 2, 1, 3)
        ).reshape(NT * P, NB8 * D).astype(mybir.dt.np(mybir.dt.float8e4))
        m = {"sd": sd_c, "ctx": ctx_c, "ctx8": ctx8_c}
        m.update(consts)
        in_maps.append(m)

    results = run_bass_kernel_spmd(nc, in_maps, core_ids=list(range(NCORES)),
                                   trace=trace, **kw)
    outs = []
    for c in range(NCORES):
        a = np.asarray(results.results[c]["out"]).astype(np.float32)
        a = a.reshape(NT // OG, P, OG, D).transpose(0, 2, 1, 3).reshape(ROWS, D)
        outs.append(a)
    out = np.concatenate(outs, axis=0)
    return out, results


def kernel(**inputs) -> np.ndarray:
    out, _ = run(inputs)
    return out
